# revision 11
# baseline (speedup 1.0000x reference)
"""CRF negative-log-likelihood loss kernel for Trainium2 (8 NeuronCores).

Reference math (per batch element b, L=512 steps, T=128 tags, mask == all-ones):
    num[b]  = start[tg0] + em[0,tg0] + sum_{i>=1} (trans[tg_{i-1},tg_i] + em[i,tg_i])
              + end[tg_{L-1}]
    logZ[b] = logsumexp over tag paths (forward algorithm)
    loss    = -mean_b(num[b] - logZ[b])

Device strategy (data-parallel, 32 sequences per core):
  Denominator: scaled forward algorithm in the *linear* domain.  With
  Etil_i = exp(em_i - kappa) and expT = exp(trans), the recursion
      A_i[t,b] = Etil_i[t,b] * sum_p expT[p,t] * A_{i-1}[p,b]
  is one 128x128 @ 128x32 matmul (stationary expT) plus one elementwise
  multiply per step.  kappa ~= E[log colsum growth] keeps A in fp32/bf16
  range for the full 512 steps without per-step renormalization (drift is a
  random walk of std ~2.6 << log(FLT_MAX)).  A forward chain from step 0 and
  a backward chain from step 511 run concurrently and meet in the middle,
  halving the serial critical path:  Z = e^{512 kappa} * sum_p A_255 * B_255.
  Both chains share each round's DVE multiply (paired PSUM columns).

  Numerator: one-hot tricks.
   - emission gather em[r, tag[r]]: scalar_tensor_tensor
     (iota == tag[r]) * em  with free-dim accumulate, one op per 128 rows.
   - transition gather: count matrix C[p,t] = #{i: tg_{i-1}=p, tg_i=t} via
     PSUM-accumulated matmuls of one-hot tiles, then <C, trans> once.
   - start/end: tiny one-hot count vectors contracted with start/end vectors.
  All partial sums land as columns of a (128, 131) fp32 buffer, contracted
  once with a ones-vector matmul at the end.

Host does only sharding/layout (transpose, dtype cast, batch slice) and the
final 8-way scalar combine:  loss = -(sum num_partial - sum logZ_partial)/B.
"""

import numpy as np

import concourse.bass as bass
import concourse.tile as tile_mod
from concourse import mybir
from concourse.bass_utils import run_bass_kernel_spmd
from concourse.vector_clock import ScopedClock
from bass_rust import SyncInfo

F32 = mybir.dt.float32
BF16 = mybir.dt.bfloat16
NPBF16 = mybir.dt.np(BF16)
ALU = mybir.AluOpType
ACTF = mybir.ActivationFunctionType
AX = mybir.AxisListType

B, L, T = 256, 512, 128
NCORES = 8
BL = B // NCORES  # 32 sequences per core
KAPPA = 5.35      # ~ E[log step growth] = log(T * E[e^N(0,1)]) for randn emissions
NROUND = 255      # paired fwd/bwd rounds; meet at A_255 / B_255
RT_TILES = (BL * L) // 128  # 128 one-hot row tiles per core
RT_GROUP = 16               # em_rt DMA batching (tiles per DMA)
NPART = 4                   # partials columns: trace(D), <C,trans>, start, end


# ---------------------------------------------------------------------------
# Workaround: this walrus build rejects >1 sem-wait on a CTRL (Drain)
# instruction ("Too many sync wait commands").  Split Tile's kernel-tail
# drain into one drain per semaphore wait.
# ---------------------------------------------------------------------------
def _split_excess_waits(nc):
    """Hoist all but one sem-wait from every instruction onto NOP carriers
    inserted immediately before it on the same engine queue."""
    def make_carrier(engine, wait):
        n = nc.engines[engine].nop(hint="ws")
        ni = n.ins
        for b in reversed(nc.main_func.blocks):
            lst = b.instructions
            if lst and lst[-1] is ni:
                lst.pop()
                break
        else:
            raise RuntimeError("waitsplit carrier not found in any block")
        ni.sync_info = SyncInfo(on_wait=[wait], on_update=[])
        return ni

    for bb in nc.main_func.blocks:
        insts = bb.instructions
        if not any(
            getattr(i, "sync_info", None) is not None and len(i.sync_info.on_wait) > 1
            for i in insts
        ):
            continue
        new = []
        for inst in insts:
            si = getattr(inst, "sync_info", None)
            if si is not None and len(si.on_wait) > 1:
                waits = list(si.on_wait)
                for w in waits[:-1]:
                    new.append(make_carrier(inst.engine, w))
                si.on_wait = waits[-1:]
                inst.sync_info = si
            new.append(inst)
        insts[:] = new


def _split_drain_and_barrier(self, tick_clock, wait_clock):
    nc = self.nc
    _split_excess_waits(nc)
    probe = nc.sync.drain()
    wait_clock.add_sem_waits(probe.ins, ScopedClock({None: tick_clock.global_clock}))
    si = probe.ins.sync_info
    waits = list(si.on_wait)
    if len(waits) > 1:
        si.on_wait = waits[:1]
        probe.ins.sync_info = si
        for w in waits[1:]:
            d = nc.sync.drain()
            d.ins.sync_info = SyncInfo(on_wait=[w], on_update=[])

    nc.all_engine_barrier()
    assert self.sems is not None
    popped = nc._tile_sem_poison_stack.pop()
    assert popped is self._sem_poison
    nc.clear_and_free_semaphores(list(self.sems.allocated().values()))
    nc.all_engine_barrier()


tile_mod.TileContext._drain_and_barrier = _split_drain_and_barrier


def _build_program() -> bass.Bass:
    nc = bass.Bass()

    # Per-core DRAM I/O.  em_pair is (T, L, BL) bf16 with L permuted to the
    # pair order [0, 511, 1, 510, ...] so each round's fwd/bwd emission
    # slices are adjacent 64-column blocks.
    d_em_pair = nc.declare_dram_parameter("em_pair", [T, L * BL], BF16, isOutput=False)
    d_em_rt = nc.declare_dram_parameter("em_rt", [BL * L, T], BF16, isOutput=False)
    d_tags = nc.declare_dram_parameter("tags_col", [128, RT_TILES], F32, isOutput=False)
    d_tagsp = nc.declare_dram_parameter("tagsp_col", [128, RT_TILES], F32, isOutput=False)
    d_tg0 = nc.declare_dram_parameter("tg0", [BL, 1], F32, isOutput=False)
    d_tgl = nc.declare_dram_parameter("tgl", [BL, 1], F32, isOutput=False)
    d_trans = nc.declare_dram_parameter("trans", [T, T], F32, isOutput=False)
    d_transT = nc.declare_dram_parameter("transT", [T, T], BF16, isOutput=False)
    d_start = nc.declare_dram_parameter("start_t", [T, 1], F32, isOutput=False)
    d_ident = nc.declare_dram_parameter("ident", [T, T], BF16, isOutput=False)
    d_end = nc.declare_dram_parameter("end_t", [T, 1], F32, isOutput=False)
    d_out = nc.declare_dram_parameter("out", [1, 2], F32, isOutput=True)

    with tile_mod.TileContext(nc) as tc:
        with (
            tc.tile_pool(name="const", bufs=1) as const,
            tc.tile_pool(name="embuf", bufs=1) as embuf,
            tc.tile_pool(name="ebuf", bufs=1) as ebuf,
            tc.tile_pool(name="pairs", bufs=3) as pairs,
            tc.tile_pool(name="rtbuf", bufs=2) as rtbuf,
            tc.tile_pool(name="ohp", bufs=4) as ohp,
            tc.tile_pool(name="parts", bufs=1) as parts,
            tc.tile_pool(name="misc", bufs=1) as misc,
            tc.tile_pool(name="pps", bufs=2, space="PSUM") as pps,
            tc.tile_pool(name="cps", bufs=1, space="PSUM") as cps,
            tc.tile_pool(name="sps", bufs=1, space="PSUM") as sps,
        ):
            # ---- constants ----
            iota_bf = const.tile([128, 128], BF16, tag="iota")
            nc.gpsimd.iota(
                iota_bf[:], pattern=[[1, 128]], base=0, channel_multiplier=0,
                allow_small_or_imprecise_dtypes=True,
            )
            ones_bf = const.tile([128, 1], BF16, tag="ones_bf")
            nc.vector.memset(ones_bf[:], 1.0)
            ones_f = const.tile([128, 1], F32, tag="ones_f")
            nc.vector.memset(ones_f[:], 1.0)

            trans_sb = const.tile([T, T], F32, tag="trans")
            nc.sync.dma_start(trans_sb[:], d_trans[:])
            transT_sb = const.tile([T, T], BF16, tag="transT")
            nc.sync.dma_start(transT_sb[:], d_transT[:])
            start_sb = const.tile([T, 1], F32, tag="startv")
            nc.sync.dma_start(start_sb[:], d_start[:])
            end_sb = const.tile([T, 1], F32, tag="endv")
            nc.sync.dma_start(end_sb[:], d_end[:])
            tags_sb = const.tile([128, RT_TILES], F32, tag="tags")
            nc.sync.dma_start(tags_sb[:], d_tags[:])
            tagsp_sb = const.tile([128, RT_TILES], F32, tag="tagsp")
            nc.sync.dma_start(tagsp_sb[:], d_tagsp[:])
            tg0_sb = const.tile([BL, 1], F32, tag="tg0")
            nc.sync.dma_start(tg0_sb[:], d_tg0[:])
            tgl_sb = const.tile([BL, 1], F32, tag="tgl")
            nc.sync.dma_start(tgl_sb[:], d_tgl[:])
            ident_sb = const.tile([T, T], BF16, tag="ident")
            nc.sync.dma_start(ident_sb[:], d_ident[:])

            expT = const.tile([T, T], BF16, tag="expT")
            nc.scalar.activation(expT[:], trans_sb[:], ACTF.Exp)
            expTT = const.tile([T, T], BF16, tag="expTT")
            nc.scalar.activation(expTT[:], transT_sb[:], ACTF.Exp)
            expS = const.tile([T, 1], F32, tag="expS")
            nc.scalar.activation(expS[:], start_sb[:], ACTF.Exp)
            expE = const.tile([T, 1], F32, tag="expE")
            nc.scalar.activation(expE[:], end_sb[:], ACTF.Exp)
            negk = const.tile([128, 1], F32, tag="negk")
            nc.vector.memset(negk[:], -KAPPA)

            # ---- emissions: DMA in chunks, exp(em - kappa) on ACT ----
            emraw = embuf.tile([T, L * BL], BF16, tag="emraw")
            etil = ebuf.tile([T, L * BL], BF16, tag="etil")
            CH = 1024  # free-dim elements per chunk = 32 L-slots
            for c in range(L * BL // CH):
                sl = slice(c * CH, (c + 1) * CH)
                nc.sync.dma_start(emraw[:, sl], d_em_pair[:, sl])
                nc.scalar.activation(etil[:, sl], emraw[:, sl], ACTF.Exp, bias=negk[:])

            partials = parts.tile([128, NPART], F32, tag="partials")
            c_psum = cps.tile([T, T], F32, tag="Cmat")
            d_psum = cps.tile([T, T], F32, tag="Dmat")

            # ---- chain init: pair_0 = [A_0 | c_511] ----
            pb = pairs.tile([128, 2 * BL], BF16, tag="pair")
            nc.vector.tensor_scalar_mul(pb[:, 0:BL], etil[:, 0:BL], expS[:])
            nc.vector.tensor_scalar_mul(pb[:, BL : 2 * BL], etil[:, BL : 2 * BL], expE[:])

            # ---- numerator tile emitter (interleaved with chain rounds) ----
            rt_tile = [None]

            def emit_numerator(k: int):
                g, sub = divmod(k, RT_GROUP)
                if sub == 0:
                    rt_tile[0] = rtbuf.tile([128, RT_GROUP * T], BF16, tag="rt", name="rt")
                    src = d_em_rt[g * RT_GROUP * 128 : (g + 1) * RT_GROUP * 128, :]
                    src = src.rearrange("(s p) t -> p s t", p=128)
                    dst = rt_tile[0][:].rearrange("p (s t) -> p s t", t=T)
                    nc.sync.dma_start(dst, src)
                ohn = ohp.tile([128, 128], BF16, tag="ohn")
                nc.gpsimd.tensor_scalar(
                    ohn[:], iota_bf[:], tags_sb[:, k : k + 1], None, ALU.is_equal
                )
                ohprev = ohp.tile([128, 128], BF16, tag="ohprev")
                nc.gpsimd.tensor_scalar(
                    ohprev[:], iota_bf[:], tagsp_sb[:, k : k + 1], None, ALU.is_equal
                )
                nc.tensor.matmul(
                    c_psum[:], ohprev[:], ohn[:],
                    start=(k == 0), stop=(k == RT_TILES - 1), skip_group_check=True,
                )
                nc.tensor.matmul(
                    d_psum[:], rt_tile[0][:, sub * T : (sub + 1) * T], ohn[:],
                    start=(k == 0), stop=(k == RT_TILES - 1), skip_group_check=True,
                )

            # ---- main loop: 255 paired fwd/bwd rounds, numerator interleaved ----
            num_k = 0
            for j in range(1, NROUND + 1):
                pp = pps.tile([128, 2 * BL], F32, tag="pairps")
                nc.tensor.matmul(pp[:, 0:BL], expT[:], pb[:, 0:BL], start=True, stop=True)
                nc.tensor.matmul(
                    pp[:, BL : 2 * BL], expTT[:], pb[:, BL : 2 * BL], start=True, stop=True
                )
                nb = pairs.tile([128, 2 * BL], BF16, tag="pair")
                nc.vector.tensor_tensor(
                    nb[:], pp[:], etil[:, 2 * j * BL : 2 * (j + 1) * BL], ALU.mult
                )
                pb = nb
                if j % 2 == 1 and num_k < RT_TILES:
                    emit_numerator(num_k)
                    num_k += 1
            while num_k < RT_TILES:
                emit_numerator(num_k)
                num_k += 1

            # ---- meet: Z*e^{-512k} = sum_p A_255 * (expTT @ c_256) ----
            bfin = sps.tile([128, BL], F32, tag="bfin")
            nc.tensor.matmul(bfin[:], expTT[:], pb[:, BL : 2 * BL], start=True, stop=True)
            dmul = misc.tile([128, BL], BF16, tag="dmul")
            nc.vector.tensor_tensor(dmul[:], bfin[:], pb[:, 0:BL], ALU.mult)
            dot = sps.tile([1, BL], F32, tag="dot")
            nc.tensor.matmul(dot[:], ones_bf[:], dmul[:], start=True, stop=True)
            logdot = misc.tile([1, BL], F32, tag="logdot")
            nc.scalar.activation(logdot[:], dot[:], ACTF.Ln)
            nc.vector.tensor_scalar_add(logdot[:], logdot[:], float(L) * KAPPA)
            resb = misc.tile([1, 2], F32, tag="resb")
            nc.vector.tensor_reduce(resb[:, 1:2], logdot[:], axis=AX.X, op=ALU.add)

            # ---- numerator epilogue ----
            tr_sc = misc.tile([T, T], F32, tag="trsc")
            nc.vector.tensor_tensor(tr_sc[:], d_psum[:], ident_sb[:], ALU.mult)
            nc.vector.tensor_reduce(
                partials[:, 0:1], tr_sc[:], axis=AX.X, op=ALU.add
            )
            ttr_sc = misc.tile([T, T], F32, tag="ttrsc")
            nc.vector.tensor_tensor(ttr_sc[:], c_psum[:], trans_sb[:], ALU.mult)
            nc.vector.tensor_reduce(
                partials[:, 1:2], ttr_sc[:], axis=AX.X, op=ALU.add
            )
            oh0 = misc.tile([BL, 128], BF16, tag="oh0")
            nc.vector.tensor_scalar(
                oh0[:], iota_bf[0:BL, :], tg0_sb[:], None, ALU.is_equal
            )
            cnt0 = sps.tile([128, 1], F32, tag="cnt")
            nc.tensor.matmul(cnt0[:], oh0[:], ones_bf[0:BL, :], start=True, stop=True)
            nc.vector.tensor_tensor(
                partials[:, 2:3], cnt0[:], start_sb[:], ALU.mult
            )
            ohl = misc.tile([BL, 128], BF16, tag="ohl")
            nc.vector.tensor_scalar(
                ohl[:], iota_bf[0:BL, :], tgl_sb[:], None, ALU.is_equal
            )
            cntl = sps.tile([128, 1], F32, tag="cnt")
            nc.tensor.matmul(cntl[:], ohl[:], ones_bf[0:BL, :], start=True, stop=True)
            nc.vector.tensor_tensor(
                partials[:, 3:4], cntl[:], end_sb[:], ALU.mult
            )

            nsum = sps.tile([1, NPART], F32, tag="nsum")
            nc.tensor.matmul(nsum[:], ones_f[:], partials[:], start=True, stop=True)
            nc.vector.tensor_reduce(resb[:, 0:1], nsum[:], axis=AX.X, op=ALU.add)

            nc.sync.dma_start(d_out[:], resb[:])

    return nc


_PAIR_ORDER = np.empty(L, dtype=np.int64)
_PAIR_ORDER[0::2] = np.arange(L // 2)
_PAIR_ORDER[1::2] = L - 1 - np.arange(L // 2)


def _prepare_in_maps(emissions, tags):
    em = np.asarray(emissions, dtype=np.float32)        # (B, L, T)
    tg = np.asarray(tags).astype(np.int64)              # (B, L)
    in_maps = []
    for c in range(NCORES):
        sl = slice(c * BL, (c + 1) * BL)
        emc = em[sl]                                    # (BL, L, T)
        em_pair = np.ascontiguousarray(
            emc.transpose(2, 1, 0)[:, _PAIR_ORDER, :]
        ).reshape(T, L * BL)
        em_rt = emc.reshape(BL * L, T)
        tf = tg[sl].reshape(-1).astype(np.float32)      # (BL*L,) b-major, i fastest
        tprev = np.roll(tf, 1)
        tprev[0::L] = -1.0
        in_maps.append({
            "ident": np.eye(T, dtype=np.float32).astype(NPBF16),
            "em_pair": em_pair.astype(NPBF16),
            "em_rt": em_rt.astype(NPBF16),
            "tags_col": np.ascontiguousarray(tf.reshape(RT_TILES, 128).T),
            "tagsp_col": np.ascontiguousarray(tprev.reshape(RT_TILES, 128).T),
            "tg0": tg[sl][:, 0:1].astype(np.float32),
            "tgl": tg[sl][:, L - 1 :].astype(np.float32),
            "trans": None,  # filled by caller
            "transT": None,
            "start_t": None,
            "end_t": None,
        })
    return in_maps


def _fill_params(in_maps, start_transitions, end_transitions, transitions):
    tr = np.asarray(transitions, dtype=np.float32)
    st = np.asarray(start_transitions, dtype=np.float32).reshape(T, 1)
    en = np.asarray(end_transitions, dtype=np.float32).reshape(T, 1)
    trT = np.ascontiguousarray(tr.T).astype(NPBF16)
    for m in in_maps:
        m["trans"] = tr
        m["transT"] = trT
        m["start_t"] = st
        m["end_t"] = en
    return in_maps


def _combine(results) -> np.float32:
    num = 0.0
    slz = 0.0
    for r in results:
        o = np.asarray(r["out"], dtype=np.float64)
        num += o[0, 0]
        slz += o[0, 1]
    return np.float32(-(num - slz) / B)


_PROGRAM = None


def _get_program():
    global _PROGRAM
    if _PROGRAM is None:
        _PROGRAM = _build_program()
    return _PROGRAM


def kernel(emissions, tags, mask, start_transitions, end_transitions, transitions,
           **_unused):
    # mask is all-ones in this problem spec (fill: ones); the kernel hardcodes
    # full-length sequences.
    nc = _get_program()
    in_maps = _fill_params(
        _prepare_in_maps(emissions, tags),
        start_transitions, end_transitions, transitions,
    )
    res = run_bass_kernel_spmd(nc, in_maps, list(range(NCORES)))
    return _combine(res.results)


# revision 16
# speedup vs baseline: 3.9389x; 3.9389x over previous
"""CRF negative-log-likelihood loss kernel for Trainium2 (8 NeuronCores).

Reference math (per batch element b, L=512 steps, T=128 tags, mask == all-ones):
    num[b]  = start[tg0] + em[0,tg0] + sum_{i>=1} (trans[tg_{i-1},tg_i] + em[i,tg_i])
              + end[tg_{L-1}]
    logZ[b] = logsumexp over tag paths (forward algorithm)
    loss    = -mean_b(num[b] - logZ[b])

Device strategy (data-parallel, 32 sequences per core):
  Denominator: scaled forward algorithm in the *linear* domain.  With
  Etil_i = exp(em_i - kappa) and expT = exp(trans), the recursion
      A_i[t,b] = Etil_i[t,b] * sum_p expT[p,t] * A_{i-1}[p,b]
  is one 128x128 @ 128x32 matmul (stationary expT) plus one elementwise
  multiply per step.  kappa ~= E[log colsum growth] keeps A in fp32/bf16
  range for the full 512 steps without per-step renormalization (drift is a
  random walk of std ~2.6 << log(FLT_MAX)).  A forward chain from step 0 and
  a backward chain from step 511 run concurrently and meet in the middle,
  halving the serial critical path:  Z = e^{512 kappa} * sum_p A_255 * B_255.
  Both chains share each round's DVE multiply (paired PSUM columns).

  Numerator: one-hot tricks.
   - emission gather em[r, tag[r]]: scalar_tensor_tensor
     (iota == tag[r]) * em  with free-dim accumulate, one op per 128 rows.
   - transition gather: count matrix C[p,t] = #{i: tg_{i-1}=p, tg_i=t} via
     PSUM-accumulated matmuls of one-hot tiles, then <C, trans> once.
   - start/end: tiny one-hot count vectors contracted with start/end vectors.
  All partial sums land as columns of a (128, 131) fp32 buffer, contracted
  once with a ones-vector matmul at the end.

Host does only sharding/layout (transpose, dtype cast, batch slice) and the
final 8-way scalar combine:  loss = -(sum num_partial - sum logZ_partial)/B.
"""

import numpy as np

import concourse.bass as bass
import concourse.tile as tile_mod
from concourse import mybir
from concourse.bass_utils import run_bass_kernel_spmd
from concourse.vector_clock import ScopedClock
from bass_rust import SyncInfo

F32 = mybir.dt.float32
BF16 = mybir.dt.bfloat16
NPBF16 = mybir.dt.np(BF16)
ALU = mybir.AluOpType
ACTF = mybir.ActivationFunctionType
AX = mybir.AxisListType

B, L, T = 256, 512, 128
NCORES = 8
BL = B // NCORES  # 32 sequences per core
KAPPA = 5.35      # ~ E[log step growth] = log(T * E[e^N(0,1)]) for randn emissions
NROUND = 255      # paired fwd/bwd rounds; meet at A_255 / B_255
RT_TILES = (BL * L) // 128  # 128 one-hot row tiles per core
RT_GROUP = 16               # em_rt DMA batching (tiles per DMA)
NPART = 4                   # partials columns: trace(D), <C,trans>, start, end


# ---------------------------------------------------------------------------
# Workaround: this walrus build rejects >1 sem-wait on a CTRL (Drain)
# instruction ("Too many sync wait commands").  Split Tile's kernel-tail
# drain into one drain per semaphore wait.
# ---------------------------------------------------------------------------
def _split_excess_waits(nc):
    """Hoist all but one sem-wait from every instruction onto NOP carriers
    inserted immediately before it on the same engine queue."""
    def make_carrier(engine, wait):
        n = nc.engines[engine].nop(hint="ws")
        ni = n.ins
        for b in reversed(nc.main_func.blocks):
            lst = b.instructions
            if lst and lst[-1] is ni:
                lst.pop()
                break
        else:
            raise RuntimeError("waitsplit carrier not found in any block")
        ni.sync_info = SyncInfo(on_wait=[wait], on_update=[])
        return ni

    for bb in nc.main_func.blocks:
        insts = bb.instructions
        if not any(
            getattr(i, "sync_info", None) is not None and len(i.sync_info.on_wait) > 1
            for i in insts
        ):
            continue
        new = []
        for inst in insts:
            si = getattr(inst, "sync_info", None)
            if si is not None and len(si.on_wait) > 1:
                waits = list(si.on_wait)
                for w in waits[:-1]:
                    new.append(make_carrier(inst.engine, w))
                si.on_wait = waits[-1:]
                inst.sync_info = si
            new.append(inst)
        insts[:] = new


def _split_drain_and_barrier(self, tick_clock, wait_clock):
    nc = self.nc
    _split_excess_waits(nc)
    probe = nc.sync.drain()
    wait_clock.add_sem_waits(probe.ins, ScopedClock({None: tick_clock.global_clock}))
    si = probe.ins.sync_info
    waits = list(si.on_wait)
    if len(waits) > 1:
        si.on_wait = waits[:1]
        probe.ins.sync_info = si
        for w in waits[1:]:
            d = nc.sync.drain()
            d.ins.sync_info = SyncInfo(on_wait=[w], on_update=[])

    nc.all_engine_barrier()
    assert self.sems is not None
    popped = nc._tile_sem_poison_stack.pop()
    assert popped is self._sem_poison
    nc.clear_and_free_semaphores(list(self.sems.allocated().values()))
    nc.all_engine_barrier()


tile_mod.TileContext._drain_and_barrier = _split_drain_and_barrier


SKIP_NUMERATOR = False


def _build_program() -> bass.Bass:
    nc = bass.Bass()

    # Per-core DRAM I/O.  em_pair is (T, L, BL) bf16 with L permuted to the
    # pair order [0, 511, 1, 510, ...] so each round's fwd/bwd emission
    # slices are adjacent 64-column blocks.
    d_em_pair = nc.declare_dram_parameter("em_pair", [T, L * BL], BF16, isOutput=False)
    d_em_rt = nc.declare_dram_parameter("em_rt", [BL * L, T], BF16, isOutput=False)
    d_tags = nc.declare_dram_parameter("tags_col", [128, RT_TILES], F32, isOutput=False)
    d_tagsp = nc.declare_dram_parameter("tagsp_col", [128, RT_TILES], F32, isOutput=False)
    d_tg0 = nc.declare_dram_parameter("tg0", [BL, 1], F32, isOutput=False)
    d_tgl = nc.declare_dram_parameter("tgl", [BL, 1], F32, isOutput=False)
    d_trans = nc.declare_dram_parameter("trans", [T, T], F32, isOutput=False)
    d_transT = nc.declare_dram_parameter("transT", [T, T], BF16, isOutput=False)
    d_start = nc.declare_dram_parameter("start_t", [T, 1], F32, isOutput=False)
    d_ident = nc.declare_dram_parameter("ident", [T, T], BF16, isOutput=False)
    d_end = nc.declare_dram_parameter("end_t", [T, 1], F32, isOutput=False)
    d_out = nc.declare_dram_parameter("out", [1, 2], F32, isOutput=True)

    with tile_mod.TileContext(nc) as tc:
        with (
            tc.tile_pool(name="const", bufs=1) as const,
            tc.tile_pool(name="embuf", bufs=1) as embuf,
            tc.tile_pool(name="ebuf", bufs=1) as ebuf,
            tc.tile_pool(name="fbuf", bufs=3) as fbuf,
            tc.tile_pool(name="bbuf", bufs=3) as bbuf,
            tc.tile_pool(name="rtbuf", bufs=2) as rtbuf,
            tc.tile_pool(name="ohp", bufs=4) as ohp,
            tc.tile_pool(name="parts", bufs=1) as parts,
            tc.tile_pool(name="misc", bufs=1) as misc,
            tc.tile_pool(name="fps", bufs=2, space="PSUM") as fps,
            tc.tile_pool(name="bps", bufs=2, space="PSUM") as bps,
            tc.tile_pool(name="cps", bufs=1, space="PSUM") as cps,
            tc.tile_pool(name="sps", bufs=2, space="PSUM") as sps,
        ):
            # ---- constants ----
            iota_bf = const.tile([128, 128], BF16, tag="iota")
            nc.gpsimd.iota(
                iota_bf[:], pattern=[[1, 128]], base=0, channel_multiplier=0,
                allow_small_or_imprecise_dtypes=True,
            )
            ones_bf = const.tile([128, 1], BF16, tag="ones_bf")
            nc.vector.memset(ones_bf[:], 1.0)
            ones_f = const.tile([128, 1], F32, tag="ones_f")
            nc.vector.memset(ones_f[:], 1.0)

            trans_sb = const.tile([T, T], F32, tag="trans")
            nc.sync.dma_start(trans_sb[:], d_trans[:])
            transT_sb = const.tile([T, T], BF16, tag="transT")
            nc.sync.dma_start(transT_sb[:], d_transT[:])
            start_sb = const.tile([T, 1], F32, tag="startv")
            nc.sync.dma_start(start_sb[:], d_start[:])
            end_sb = const.tile([T, 1], F32, tag="endv")
            nc.sync.dma_start(end_sb[:], d_end[:])
            tags_sb = const.tile([128, RT_TILES], F32, tag="tags")
            nc.sync.dma_start(tags_sb[:], d_tags[:])
            tagsp_sb = const.tile([128, RT_TILES], F32, tag="tagsp")
            nc.sync.dma_start(tagsp_sb[:], d_tagsp[:])
            tg0_sb = const.tile([BL, 1], F32, tag="tg0")
            nc.sync.dma_start(tg0_sb[:], d_tg0[:])
            tgl_sb = const.tile([BL, 1], F32, tag="tgl")
            nc.sync.dma_start(tgl_sb[:], d_tgl[:])
            ident_sb = const.tile([T, T], BF16, tag="ident")
            nc.sync.dma_start(ident_sb[:], d_ident[:])

            expT = const.tile([T, T], BF16, tag="expT")
            nc.scalar.activation(expT[:], trans_sb[:], ACTF.Exp)
            expTT = const.tile([T, T], BF16, tag="expTT")
            nc.scalar.activation(expTT[:], transT_sb[:], ACTF.Exp)
            expS = const.tile([T, 1], F32, tag="expS")
            nc.scalar.activation(expS[:], start_sb[:], ACTF.Exp)
            expE = const.tile([T, 1], F32, tag="expE")
            nc.scalar.activation(expE[:], end_sb[:], ACTF.Exp)
            negk = const.tile([128, 1], F32, tag="negk")
            nc.vector.memset(negk[:], -KAPPA)

            # ---- emissions: DMA in chunks, exp(em - kappa) on ACT ----
            emraw = embuf.tile([T, L * BL], BF16, tag="emraw")
            etil = ebuf.tile([T, L * BL], BF16, tag="etil")
            CH = 1024  # free-dim elements per chunk = 32 L-slots
            for c in range(L * BL // CH):
                sl = slice(c * CH, (c + 1) * CH)
                nc.sync.dma_start(emraw[:, sl], d_em_pair[:, sl])
                nc.scalar.activation(etil[:, sl], emraw[:, sl], ACTF.Exp, bias=negk[:])

            partials = parts.tile([128, NPART], F32, tag="partials")
            if not SKIP_NUMERATOR:
                c_psum = cps.tile([T, T], F32, tag="Cmat")
                d_psum = cps.tile([T, T], F32, tag="Dmat")

            # ---- chain init: A_0 (fwd) and c_511 (bwd) ----
            fa = fbuf.tile([128, BL], BF16, tag="fa", name="fa")
            nc.vector.tensor_scalar_mul(fa[:], etil[:, 0:BL], expS[:])
            bc = bbuf.tile([128, BL], BF16, tag="bc", name="bc")
            nc.vector.tensor_scalar_mul(bc[:], etil[:, BL : 2 * BL], expE[:])

            # ---- numerator tile emitter (interleaved with chain rounds) ----
            rt_tile = [None]

            def emit_numerator(k: int):
                g, sub = divmod(k, RT_GROUP)
                if sub == 0:
                    rt_tile[0] = rtbuf.tile([128, RT_GROUP * T], BF16, tag="rt", name="rt")
                    src = d_em_rt[g * RT_GROUP * 128 : (g + 1) * RT_GROUP * 128, :]
                    src = src.rearrange("(s p) t -> p s t", p=128)
                    dst = rt_tile[0][:].rearrange("p (s t) -> p s t", t=T)
                    nc.sync.dma_start(dst, src)
                ohn = ohp.tile([128, 128], BF16, tag="ohn")
                nc.vector.tensor_scalar(
                    ohn[:], iota_bf[:], tags_sb[:, k : k + 1], None, ALU.is_equal
                )
                ohprev = ohp.tile([128, 128], BF16, tag="ohprev")
                nc.vector.tensor_scalar(
                    ohprev[:], iota_bf[:], tagsp_sb[:, k : k + 1], None, ALU.is_equal
                )
                nc.tensor.matmul(
                    c_psum[:], ohprev[:], ohn[:],
                    start=(k == 0), stop=(k == RT_TILES - 1), skip_group_check=True,
                )
                nc.tensor.matmul(
                    d_psum[:], rt_tile[0][:, sub * T : (sub + 1) * T], ohn[:],
                    start=(k == 0), stop=(k == RT_TILES - 1), skip_group_check=True,
                )

            # ---- main loop: 255 paired fwd/bwd rounds, numerator interleaved ----
            num_k = 0
            for j in range(1, NROUND + 1):
                fp = fps.tile([128, BL], F32, tag="fp", name="fp")
                nc.tensor.matmul(fp[:], expT[:], fa[:], start=True, stop=True)
                nfa = fbuf.tile([128, BL], BF16, tag="fa", name="nfa")
                nc.vector.tensor_tensor(
                    nfa[:], fp[:], etil[:, 2 * j * BL : (2 * j + 1) * BL], ALU.mult
                )
                fa = nfa
                bp = bps.tile([128, BL], F32, tag="bp", name="bp")
                nc.tensor.matmul(bp[:], expTT[:], bc[:], start=True, stop=True)
                nbc = bbuf.tile([128, BL], BF16, tag="bc", name="nbc")
                nc.vector.tensor_tensor(
                    nbc[:], bp[:], etil[:, (2 * j + 1) * BL : (2 * j + 2) * BL], ALU.mult
                )
                bc = nbc
                if not SKIP_NUMERATOR and j % 2 == 1 and num_k < RT_TILES:
                    emit_numerator(num_k)
                    num_k += 1
            while not SKIP_NUMERATOR and num_k < RT_TILES:
                emit_numerator(num_k)
                num_k += 1

            # ---- meet: Z*e^{-512k} = sum_p A_255 * (expTT @ c_256) ----
            bfin = sps.tile([128, BL], F32, tag="fin", name="bfin")
            nc.tensor.matmul(bfin[:], expTT[:], bc[:], start=True, stop=True)
            dmul = misc.tile([128, BL], BF16, tag="dmul")
            nc.vector.tensor_tensor(dmul[:], bfin[:], fa[:], ALU.mult)
            dot = sps.tile([1, BL], F32, tag="fin", name="dot")
            nc.tensor.matmul(dot[:], ones_bf[:], dmul[:], start=True, stop=True)
            logdot = misc.tile([1, BL], F32, tag="logdot")
            nc.scalar.activation(logdot[:], dot[:], ACTF.Ln)
            nc.vector.tensor_scalar_add(logdot[:], logdot[:], float(L) * KAPPA)
            resb = misc.tile([1, 2], F32, tag="resb")
            nc.vector.tensor_reduce(resb[:, 1:2], logdot[:], axis=AX.X, op=ALU.add)

            # ---- numerator epilogue ----
            if SKIP_NUMERATOR:
                nc.vector.memset(partials[:, 0:2], 0.0)
            else:
                tr_sc = misc.tile([T, T], F32, tag="trsc")
                nc.vector.tensor_tensor(tr_sc[:], d_psum[:], ident_sb[:], ALU.mult)
                nc.vector.tensor_reduce(
                    partials[:, 0:1], tr_sc[:], axis=AX.X, op=ALU.add
                )
                ttr_sc = misc.tile([T, T], F32, tag="ttrsc")
                nc.vector.tensor_tensor(ttr_sc[:], c_psum[:], trans_sb[:], ALU.mult)
                nc.vector.tensor_reduce(
                    partials[:, 1:2], ttr_sc[:], axis=AX.X, op=ALU.add
                )
            oh0 = misc.tile([BL, 128], BF16, tag="oh0")
            nc.vector.tensor_scalar(
                oh0[:], iota_bf[0:BL, :], tg0_sb[:], None, ALU.is_equal
            )
            cnt0 = sps.tile([128, 1], F32, tag="fin", name="cnt0")
            nc.tensor.matmul(cnt0[:], oh0[:], ones_bf[0:BL, :], start=True, stop=True)
            nc.vector.tensor_tensor(
                partials[:, 2:3], cnt0[:], start_sb[:], ALU.mult
            )
            ohl = misc.tile([BL, 128], BF16, tag="ohl")
            nc.vector.tensor_scalar(
                ohl[:], iota_bf[0:BL, :], tgl_sb[:], None, ALU.is_equal
            )
            cntl = sps.tile([128, 1], F32, tag="fin", name="cntl")
            nc.tensor.matmul(cntl[:], ohl[:], ones_bf[0:BL, :], start=True, stop=True)
            nc.vector.tensor_tensor(
                partials[:, 3:4], cntl[:], end_sb[:], ALU.mult
            )

            nsum = sps.tile([1, NPART], F32, tag="fin", name="nsum")
            nc.tensor.matmul(nsum[:], ones_f[:], partials[:], start=True, stop=True)
            nc.vector.tensor_reduce(resb[:, 0:1], nsum[:], axis=AX.X, op=ALU.add)

            nc.sync.dma_start(d_out[:], resb[:])

    return nc


_PAIR_ORDER = np.empty(L, dtype=np.int64)
_PAIR_ORDER[0::2] = np.arange(L // 2)
_PAIR_ORDER[1::2] = L - 1 - np.arange(L // 2)


def _prepare_in_maps(emissions, tags):
    em = np.asarray(emissions, dtype=np.float32)        # (B, L, T)
    tg = np.asarray(tags).astype(np.int64)              # (B, L)
    in_maps = []
    for c in range(NCORES):
        sl = slice(c * BL, (c + 1) * BL)
        emc = em[sl]                                    # (BL, L, T)
        em_pair = np.ascontiguousarray(
            emc.transpose(2, 1, 0)[:, _PAIR_ORDER, :]
        ).reshape(T, L * BL)
        em_rt = emc.reshape(BL * L, T)
        tf = tg[sl].reshape(-1).astype(np.float32)      # (BL*L,) b-major, i fastest
        tprev = np.roll(tf, 1)
        tprev[0::L] = -1.0
        in_maps.append({
            "ident": np.eye(T, dtype=np.float32).astype(NPBF16),
            "em_pair": em_pair.astype(NPBF16),
            "em_rt": em_rt.astype(NPBF16),
            "tags_col": np.ascontiguousarray(tf.reshape(RT_TILES, 128).T),
            "tagsp_col": np.ascontiguousarray(tprev.reshape(RT_TILES, 128).T),
            "tg0": tg[sl][:, 0:1].astype(np.float32),
            "tgl": tg[sl][:, L - 1 :].astype(np.float32),
            "trans": None,  # filled by caller
            "transT": None,
            "start_t": None,
            "end_t": None,
        })
    return in_maps


def _fill_params(in_maps, start_transitions, end_transitions, transitions):
    tr = np.asarray(transitions, dtype=np.float32)
    st = np.asarray(start_transitions, dtype=np.float32).reshape(T, 1)
    en = np.asarray(end_transitions, dtype=np.float32).reshape(T, 1)
    trT = np.ascontiguousarray(tr.T).astype(NPBF16)
    for m in in_maps:
        m["trans"] = tr
        m["transT"] = trT
        m["start_t"] = st
        m["end_t"] = en
    return in_maps


def _combine(results) -> np.float32:
    num = 0.0
    slz = 0.0
    for r in results:
        o = np.asarray(r["out"], dtype=np.float64)
        num += o[0, 0]
        slz += o[0, 1]
    return np.float32(-(num - slz) / B)


_PROGRAM = None


def _get_program():
    global _PROGRAM
    if _PROGRAM is None:
        _PROGRAM = _build_program()
    return _PROGRAM


def kernel(emissions, tags, mask, start_transitions, end_transitions, transitions,
           **_unused):
    # mask is all-ones in this problem spec (fill: ones); the kernel hardcodes
    # full-length sequences.
    nc = _get_program()
    in_maps = _fill_params(
        _prepare_in_maps(emissions, tags),
        start_transitions, end_transitions, transitions,
    )
    res = run_bass_kernel_spmd(nc, in_maps, list(range(NCORES)))
    return _combine(res.results)
